# revision 29
# baseline (speedup 1.0000x reference)
"""GNN message-passing kernel for 8 Trainium2 NeuronCores.

Strategy (node-sharded, zero collectives):
  - Pad nodes to 50176 = 8 cores x 49 tiles x 128 slots. A host-side
    "snake deal" permutation assigns nodes to (tile, partition) slots so
    that per-tile edge counts are balanced (the MLP is pointwise, so any
    node permutation is legal; it is undone on the host at the end).
  - Edges are duplicated per direction: mi keyed by dst (gather x[src]),
    mo keyed by src (gather x[dst]). Each direction's edges are bucketed
    by owning tile and by gather-index half (dma_gather indices are
    int16, so x is split into two 25088-row tables), padded to H blocks
    of 128 edges per (tile, half) cell.
  - On-core: bulk dma_gather brings x rows for a group of tiles into
    SBUF; for each 128-edge block a one-hot selection matrix
    S[i, j] = e_i * (part(key_i) == j) is built in one DVE tensor_scalar
    op from an iota constant; PE accumulates psum[d, node] += Y^T @ S
    over the tile's blocks (Y = gathered rows, K=edges contraction).
  - The 4-layer MLP runs feature-major: h = tanh(W^T h + b) via PE
    matmuls with per-partition bias+tanh on the scalar engine. Output is
    written feature-major [128, 6272] per core and re-permuted on host.
"""

import os
import sys

sys.path.insert(0, "/opt/trn_rl_repo")

import numpy as np

from concourse import bass, bacc, mybir, tile
from concourse import bass_utils

N = 50000
E = 800000
D = 128
N_CORES = 8
T_CORE = 49                      # tiles per core
T_TOT = N_CORES * T_CORE         # 392 tiles
NPAD = T_TOT * 128               # 50176
HALF = NPAD // 2                 # 25088 (int16 index limit is 32767)
G = 2                            # tiles per gather group

f32 = mybir.dt.float32
bf16 = mybir.dt.bfloat16
i16 = mybir.dt.int16

import ml_dtypes

def _bf(a):
    return np.asarray(a, np.float32).astype(ml_dtypes.bfloat16)

LAST_RESULTS = None              # BassKernelResults of the last run


def _register_ntff_hook():
    """Make trace=True work under axon by registering the NTFF profile
    hook that the agent image's antenv package lacks."""
    import types, ctypes, contextlib

    if "antenv.axon_hooks" in sys.modules:
        return
    so_path = "/opt/axon/libaxon_pjrt.so"
    if not os.path.exists(so_path):
        return
    try:
        lib = ctypes.CDLL(so_path)
        if not hasattr(lib, "axon_start_nrt_profile"):
            return
        lib.axon_start_nrt_profile.argtypes = [
            ctypes.POINTER(ctypes.c_int64), ctypes.c_size_t]
        lib.axon_start_nrt_profile.restype = ctypes.c_int64
        lib.axon_stop_nrt_profile.argtypes = [ctypes.c_char_p]
        lib.axon_stop_nrt_profile.restype = ctypes.c_int64

        @contextlib.contextmanager
        def _hook(output_dir, device_ids):
            import jax
            jax.devices()
            if device_ids:
                ids = (ctypes.c_int64 * len(device_ids))(*device_ids)
                rc = lib.axon_start_nrt_profile(ids, len(device_ids))
            else:
                rc = lib.axon_start_nrt_profile(None, 0)
            if rc != 0:
                raise RuntimeError(f"axon_start_nrt_profile rc={rc}")
            try:
                yield
            finally:
                n = lib.axon_stop_nrt_profile(str(output_dir).encode())
                print(f"profile: {n} file(s) -> {output_dir}", file=sys.stderr)

        mod = types.ModuleType("antenv.axon_hooks")
        mod.get_axon_ntff_profile_hook = lambda: _hook
        sys.modules["antenv.axon_hooks"] = mod
    except OSError:
        pass


def _snake_slots(src, dst):
    """Assign each padded node to a (global tile, partition) slot via a
    4-D greedy LPT + swap repair, balancing all four per-(tile, half, dir)
    edge-cell counts so the global max cell stays <= 8*128 (H=8)."""
    c = np.zeros((NPAD, 4), np.int64)
    np.add.at(c[:, 0], dst[src < HALF], 1)
    np.add.at(c[:, 1], dst[src >= HALF], 1)
    np.add.at(c[:, 2], src[dst < HALF], 1)
    np.add.at(c[:, 3], src[dst >= HALF], 1)
    tot = c.sum(1)
    order = np.argsort(-tot, kind="stable")
    nz = order[tot[order] > 0]
    zz = order[tot[order] == 0]

    L = np.zeros((T_TOT, 4), np.int64)
    cnt = np.zeros(T_TOT, np.int64)
    gtile = np.empty(NPAD, np.int32)
    gpart = np.empty(NPAD, np.int32)
    for n in nz:
        v = c[n]
        cand = L + v[None, :]
        score = cand.max(1) * 4096 + cand.sum(1)
        score[cnt >= 128] = np.iinfo(np.int64).max
        t = int(np.argmin(score))
        gtile[n] = t
        gpart[n] = cnt[t]
        L[t] += v
        cnt[t] += 1
    # zero-degree / pad nodes fill remaining slots
    free_t = np.repeat(np.arange(T_TOT), 128 - cnt).astype(np.int32)
    pos = np.concatenate([np.arange(cnt[t], 128) for t in range(T_TOT)])
    assert len(free_t) == len(zz)
    gtile[zz] = free_t
    gpart[zz] = pos.astype(np.int32)

    # swap repair toward max cell <= 1024
    members = [list(np.flatnonzero(gtile == t)) for t in range(T_TOT)]

    def do_swap(t_bad, t_good, i, j):
        nb, ng = members[t_bad], members[t_good]
        n_i, n_j = nb[i], ng[j]
        L[t_bad] += c[n_j] - c[n_i]
        L[t_good] += c[n_i] - c[n_j]
        nb[i], ng[j] = n_j, n_i
        gtile[n_i], gtile[n_j] = t_good, t_bad
        gpart[n_i], gpart[n_j] = gpart[n_j], gpart[n_i]

    for _ in range(6000):
        mx = int(L.max())
        if mx <= 1024:
            break
        t_bad, d = np.unravel_index(int(np.argmax(L)), L.shape)
        cb = c[members[t_bad]]
        i_cands = np.argsort(-cb[:, d])[:12]
        goods = np.argsort(L[:, d])[:12]
        best = None
        for t_good in goods:
            cg = c[members[int(t_good)]]
            for i in i_cands:
                delta = cb[int(i)] - cg  # [128, 4]
                new_bad = (L[t_bad] - delta).max(1)
                new_good = (L[int(t_good)] + delta).max(1)
                newmx = np.maximum(new_bad, new_good)
                j = int(np.argmin(newmx))
                val = (int(newmx[j]),
                       int(new_bad[j] + new_good[j]))
                if best is None or val < best[0]:
                    best = (val, int(t_good), int(i), j)
        if best is None or best[0][0] >= mx:
            break
        do_swap(t_bad, best[1], best[2], best[3])
    return gtile, gpart


def _build_dir(key, gat, ew, gtile, gpart, H):
    """Bucket one direction's edges into padded per-(tile, half) cells.

    Returns (gidx [T_TOT, 2, H, 128] int16, ce [T_TOT, 128, 4H] f32)
    where ce columns are [c (2H) | e (2H)] with block col j = half*H+jj.
    """
    t = gtile[key]
    half = (gat >= HALF).astype(np.int64)
    cell = t.astype(np.int64) * 2 + half
    order = np.argsort(cell, kind="stable")
    cell_s = cell[order]
    cnt = np.bincount(cell_s, minlength=T_TOT * 2)
    assert cnt.max() <= H * 128, (cnt.max(), H * 128)
    starts = np.zeros(T_TOT * 2, np.int64)
    starts[1:] = np.cumsum(cnt)[:-1]
    pos = np.arange(len(key)) - starts[cell_s]
    slot = cell_s * (H * 128) + pos

    gidx = np.zeros(T_TOT * 2 * H * 128, np.int16)
    gidx[slot] = (gat[order] - half[order] * HALF).astype(np.int16)
    epad = np.zeros(T_TOT * 2 * H * 128, np.float32)
    epad[slot] = ew[order]
    cpad = np.zeros(T_TOT * 2 * H * 128, np.float32)
    cpad[slot] = gpart[key][order].astype(np.float32)

    gidx = gidx.reshape(T_TOT, 2, H, 128)
    # block col j = half*H + jj, partition p = edge index within block
    c_t = cpad.reshape(T_TOT, 2, H, 128).transpose(0, 3, 1, 2).reshape(
        T_TOT, 128, 2 * H)
    e_t = epad.reshape(T_TOT, 2, H, 128).transpose(0, 3, 1, 2).reshape(
        T_TOT, 128, 2 * H)
    ce = np.concatenate([c_t, e_t], axis=2)  # [T_TOT, 128, 4H]
    return gidx, ce


def _wrap_idx(arr):
    """[L] int16 -> [128, L//16] in the dma_gather layout: idx i at
    [i % 16, i // 16], replicated across the 8 Q7 core stripes."""
    L = arr.shape[0]
    w = arr.reshape(L // 16, 16).T  # [16, L//16]
    return np.ascontiguousarray(np.tile(w, (8, 1)))


def _preprocess(x, e, edge_index):
    src = np.asarray(edge_index[0], np.int64)
    dst = np.asarray(edge_index[1], np.int64)
    ew = np.asarray(e, np.float32)
    xpad = np.zeros((NPAD, D), np.float32)
    xpad[:N] = np.asarray(x, np.float32)

    gtile, gpart = _snake_slots(src, dst)

    # one H for the whole (uniform SPMD) program
    def _max_cell(key, gat):
        cell = gtile[key].astype(np.int64) * 2 + (gat >= HALF)
        return np.bincount(cell, minlength=T_TOT * 2).max()

    H = int(np.ceil(max(_max_cell(dst, src), _max_cell(src, dst)) / 128))

    gidx_mi, ce_mi = _build_dir(dst, src, ew, gtile, gpart, H)
    gidx_mo, ce_mo = _build_dir(src, dst, ew, gtile, gpart, H)
    ce = np.ascontiguousarray(
        np.concatenate([ce_mi, ce_mo], axis=2))  # [T_TOT, 128, 8H]

    # feature-major x in slot order for the MLP concat input
    perm_nodes = np.empty(NPAD, np.int64)
    gslot = gtile.astype(np.int64) * 128 + gpart
    perm_nodes[gslot] = np.arange(NPAD)
    xpermT = np.ascontiguousarray(xpad[perm_nodes].T)  # [128, NPAD]

    iota = _bf(np.broadcast_to(np.arange(128, dtype=np.float32),
                               (128, 128)).copy())

    x_lo_b = _bf(xpad[:HALF])
    x_hi_b = _bf(xpad[HALF:])
    per_core = []
    for k in range(N_CORES):
        ts = slice(k * T_CORE, (k + 1) * T_CORE)
        m = {
            "x_lo": x_lo_b,
            "x_hi": x_hi_b,
            "xT": _bf(np.ascontiguousarray(
                xpermT[:, k * T_CORE * 128:(k + 1) * T_CORE * 128])),
            "ce": np.ascontiguousarray(ce[ts]),
            "iota": iota,
        }
        for dname, gi in (("mi", gidx_mi), ("mo", gidx_mo)):
            for h in (0, 1):
                flat = gi[ts, h].reshape(-1)  # [T_CORE*H*128]
                m[f"idx_{dname}{h}"] = _wrap_idx(flat)
        per_core.append(m)
    return per_core, gslot, H


_NC_CACHE = {}


def _build_nc(H):
    if H in _NC_CACHE:
        return _NC_CACHE[H]
    HB = 2 * H  # blocks per (tile, direction)
    nc = bacc.Bacc("TRN2", target_bir_lowering=False, debug=False,
                   enable_asserts=False, num_devices=N_CORES,
                   num_swdge_queues=4)

    x_lo = nc.dram_tensor("x_lo", [HALF, D], bf16, kind="ExternalInput").ap()
    x_hi = nc.dram_tensor("x_hi", [HALF, D], bf16, kind="ExternalInput").ap()
    xT = nc.dram_tensor("xT", [128, T_CORE * 128], bf16,
                        kind="ExternalInput").ap()
    ce = nc.dram_tensor("ce", [T_CORE, 128, 8 * H], f32,
                        kind="ExternalInput").ap()
    iota_d = nc.dram_tensor("iota", [128, 128], bf16,
                            kind="ExternalInput").ap()
    idx = {}
    for dname in ("mi", "mo"):
        for h in (0, 1):
            idx[(dname, h)] = nc.dram_tensor(
                f"idx_{dname}{h}", [128, T_CORE * H * 8], i16,
                kind="ExternalInput").ap()
    w1 = nc.dram_tensor("W1", [3 * D, D], bf16, kind="ExternalInput").ap()
    wds = {2: nc.dram_tensor("W2", [D, D], bf16, kind="ExternalInput").ap(),
           3: nc.dram_tensor("W3", [D, D], bf16, kind="ExternalInput").ap(),
           4: nc.dram_tensor("W4", [D, D], bf16, kind="ExternalInput").ap()}
    bds = {i: nc.dram_tensor(f"b{i}", [D], f32, kind="ExternalInput").ap()
           for i in (1, 2, 3, 4)}
    out_t = nc.dram_tensor("out_t", [128, T_CORE * 128], f32,
                           kind="ExternalOutput").ap()

    eq = mybir.AluOpType.is_equal
    mul = mybir.AluOpType.mult
    tanh = mybir.ActivationFunctionType.Tanh

    with tile.TileContext(nc) as tc:
        with (
            tc.tile_pool(name="const", bufs=1) as cpool,
            tc.tile_pool(name="gath", bufs=6) as gpool,
            tc.tile_pool(name="idxp", bufs=6) as ipool,
            tc.tile_pool(name="work", bufs=3) as wpool,
            tc.tile_pool(name="sel", bufs=6) as spool,
            tc.tile_pool(name="hbuf", bufs=3) as hpool,
            tc.tile_pool(name="ps", bufs=4, space="PSUM") as pspool,
            tc.tile_pool(name="psm", bufs=2, space="PSUM") as mpool,
        ):
            # Gathers run as rolling 16-block (2048-idx) chunks; bigger
            # chunks amortize the ~1us fixed SWDGE desc-gen cost per call on
            # GpSimd (994ns + 0.34ns/desc). >1024 idxs per call needs
            # single_packet=False. Queues rotate round-robin over all 4.
            CHUNK = 16                     # blocks per gather call
            NBLK = T_CORE * H              # blocks per (dir, half) stream
            streams = [("mi", 0), ("mi", 1), ("mo", 0), ("mo", 1)]
            chunks = {s: [] for s in streams}   # chunk tiles per stream
            next_chunk = {s: 0 for s in streams}
            qrr = [0]

            def emit_chunks(upto_block):
                for s in streams:
                    dname, h = s
                    while (next_chunk[s] * CHUNK < upto_block
                           and next_chunk[s] * CHUNK < NBLK):
                        c = next_chunk[s]
                        nb = min(CHUNK, NBLK - c * CHUNK)
                        nidx = nb * 128
                        it = ipool.tile([128, nb * 8], i16,
                                        tag=f"i{dname}{h}",
                                        name=f"i{dname}{h}")
                        nc.sync.dma_start(
                            out=it[:],
                            in_=idx[s][:, c * CHUNK * 8:
                                       (c * CHUNK + nb) * 8])
                        gb = gpool.tile([128, nb, 128], bf16,
                                        tag=f"g{dname}{h}",
                                        name=f"g{dname}{h}")
                        q = (qrr[0] + 1) % 4   # rotate 1,2,3,0,...
                        qrr[0] = q
                        nc.gpsimd.dma_gather(
                            out_ap=gb[:],
                            in_ap=(x_lo if h == 0 else x_hi)[:, :],
                            idxs_ap=it[:],
                            num_idxs=nidx,
                            num_idxs_reg=nidx,
                            elem_size=D,
                            single_packet=(nidx <= 1024),
                            queue_num=q,
                        )
                        chunks[s].append(gb)
                        next_chunk[s] += 1

            # Warmup: a minimal dma_gather issued first absorbs the one-time
            # Q7 extended-inst library load (~10us) while the constant DMAs
            # stream in; without it the first real gather pays that cost.
            warm_idx = ipool.tile([128, 8], i16, tag="warm", name="warm_idx")
            nc.gpsimd.memset(warm_idx[:], 0)
            warm_out = gpool.tile([128, 1, 128], bf16, tag="warm",
                                  name="warm_out")
            nc.gpsimd.dma_gather(
                out_ap=warm_out[:], in_ap=x_lo[:, :], idxs_ap=warm_idx[:],
                num_idxs=128, num_idxs_reg=128, elem_size=D,
                single_packet=True, queue_num=0)

            # first gathers enter the queues before the constant loads
            emit_chunks(2 * H)

            iota_t = cpool.tile([128, 128], bf16)
            nc.sync.dma_start(out=iota_t[:], in_=iota_d[:, :])
            wt = {}
            for j in range(3):
                wt[(1, j)] = cpool.tile([128, 128], bf16, tag=f"w1{j}",
                                        name=f"w1{j}")
                nc.sync.dma_start(out=wt[(1, j)][:],
                                  in_=w1[j * 128:(j + 1) * 128, :])
            for i in (2, 3, 4):
                wt[i] = cpool.tile([128, 128], bf16, tag=f"w{i}",
                                   name=f"w{i}")
                nc.sync.dma_start(out=wt[i][:], in_=wds[i][:, :])
            bt = {}
            for i in (1, 2, 3, 4):
                bt[i] = cpool.tile([128, 1], f32, tag=f"b{i}",
                                   name=f"b{i}")
                nc.sync.dma_start(out=bt[i][:], in_=bds[i][:, None])

            if True:
                for t in range(T_CORE):
                    emit_chunks(min((t + 3) * H, NBLK))
                    cet = wpool.tile([128, 8 * H], f32, tag="ce")
                    nc.sync.dma_start(out=cet[:], in_=ce[t])
                    xt_t = wpool.tile([128, 128], bf16, tag="xt")
                    nc.sync.dma_start(
                        out=xt_t[:], in_=xT[:, t * 128:(t + 1) * 128])

                    acc = {}
                    for di, dname in enumerate(("mi", "mo")):
                        ps = pspool.tile([128, 128], f32, tag="scat")
                        cbase = di * 4 * H
                        ebase = cbase + 2 * H
                        for j in range(HB):
                            h = 0 if j < H else 1
                            jj = j - h * H
                            s_t = spool.tile([128, 128], bf16, tag="s")
                            nc.vector.scalar_tensor_tensor(
                                s_t[:], iota_t[:],
                                cet[:, cbase + j:cbase + j + 1],
                                cet[:, ebase + j:ebase + j + 1]
                                .to_broadcast([128, 128]),
                                eq, mul)
                            blk = t * H + jj
                            y = chunks[(dname, h)][blk // CHUNK][
                                :, blk % CHUNK, :]
                            nc.tensor.matmul(
                                out=ps[:], lhsT=y, rhs=s_t[:],
                                start=(j == 0), stop=(j == HB - 1))
                        acc[dname] = hpool.tile([128, 128], bf16,
                                                tag=f"acc{dname}",
                                                name=f"acc{dname}")
                        nc.scalar.copy(out=acc[dname][:], in_=ps[:])

                    hp = mpool.tile([128, 128], f32, tag="mlp")
                    nc.tensor.matmul(out=hp[:], lhsT=wt[(1, 0)][:],
                                     rhs=acc["mi"][:], start=True, stop=False)
                    nc.tensor.matmul(out=hp[:], lhsT=wt[(1, 1)][:],
                                     rhs=acc["mo"][:], start=False, stop=False)
                    nc.tensor.matmul(out=hp[:], lhsT=wt[(1, 2)][:],
                                     rhs=xt_t[:], start=False, stop=True)
                    hprev = hpool.tile([128, 128], bf16, tag="h")
                    nc.scalar.activation(hprev[:], hp[:], tanh,
                                         bias=bt[1][:, 0:1])
                    for i in (2, 3, 4):
                        hp = mpool.tile([128, 128], f32, tag="mlp")
                        nc.tensor.matmul(out=hp[:], lhsT=wt[i][:],
                                         rhs=hprev[:], start=True, stop=True)
                        hnext = hpool.tile([128, 128], f32 if i == 4 else bf16,
                                           tag="h4" if i == 4 else "h")
                        nc.scalar.activation(hnext[:], hp[:], tanh,
                                             bias=bt[i][:, 0:1])
                        hprev = hnext
                    nc.sync.dma_start(
                        out=out_t[:, t * 128:(t + 1) * 128], in_=hprev[:])

    nc.compile()
    _NC_CACHE[H] = nc
    return nc


def kernel(**inputs):
    global LAST_RESULTS
    _register_ntff_hook()
    x = np.asarray(inputs["x"], np.float32)
    e = np.asarray(inputs["e"], np.float32)
    edge_index = np.asarray(inputs["edge_index"])

    per_core, gslot, H = _preprocess(x, e, edge_index)
    nc = _build_nc(H)

    shared = {"W1": _bf(inputs["W1"])}
    for i in (2, 3, 4):
        shared[f"W{i}"] = _bf(inputs[f"W{i}"])
    for i in (1, 2, 3, 4):
        shared[f"b{i}"] = np.asarray(inputs[f"b{i}"], np.float32)

    in_maps = []
    for k in range(N_CORES):
        m = dict(per_core[k])
        m.update(shared)
        in_maps.append(m)

    res = bass_utils.run_bass_kernel_spmd(nc, in_maps,
                                          core_ids=list(range(N_CORES)))
    LAST_RESULTS = res
    big = np.concatenate([res.results[k]["out_t"] for k in range(N_CORES)],
                         axis=1)  # [128, NPAD] feature-major, slot order
    out = big.T[gslot[:N]]
    return np.ascontiguousarray(out.astype(np.float32))

